# revision 7
# baseline (speedup 1.0000x reference)
"""AttentivePoolingNetwork Trainium2 kernel.

Data-parallel over batch across 8 NeuronCores (64 batch elements each).
Per batch element, fully fused on-chip:
  gather bf16 emb rows -> PE-transpose to [E, L] -> conv1d(k=3) as shifted
  matmuls (bias folded in via ones-row) -> QT/AT [tokens, F] -> transposes
  -> H = U^T Q -> G = H^T A -> row/col maxes -> exp(tanh(max)) weights
  (softmax denominators cancel in the final cosine similarity) -> pooled
  rQ/rA via tiny matmuls -> cosine similarity per element.
"""

import os
import numpy as np
import ml_dtypes

import concourse.bacc as bacc
import concourse.bass as bass
import concourse.tile as tile
import concourse.mybir as mybir
from concourse import bass_utils
from concourse.masks import make_identity

BF16 = mybir.dt.bfloat16
F32 = mybir.dt.float32
I32 = mybir.dt.int32
AX = mybir.AxisListType.X
AF = mybir.ActivationFunctionType

B, QL, AL = 512, 128, 512
V1, E, F = 50001, 300, 400
NCORES = 8
BL = int(os.environ.get("KBL", B // NCORES))  # batch elems per core
EP = 320   # emb width padded (300 -> 320), bf16 rows = 640B
FP = 512   # feature width padded (400 -> 512)


def build_kernel(nc):
    emb = nc.dram_tensor("emb", [V1, EP], BF16, kind="ExternalInput").ap()
    qidx = nc.dram_tensor("qidx", [128, BL], I32, kind="ExternalInput").ap()
    aidx = nc.dram_tensor("aidx", [128, 4 * BL], I32, kind="ExternalInput").ap()
    wc0 = nc.dram_tensor("wc0", [128, 1200], BF16, kind="ExternalInput").ap()
    wc1 = nc.dram_tensor("wc1", [128, 1200], BF16, kind="ExternalInput").ap()
    wc2 = nc.dram_tensor("wc2", [65, 1200], BF16, kind="ExternalInput").ap()
    u_s = nc.dram_tensor("u_s", [128, 2048], BF16, kind="ExternalInput").ap()
    out_d = nc.dram_tensor("out", [BL], F32, kind="ExternalOutput").ap()

    with tile.TileContext(nc) as tc:
        with (
            tc.tile_pool(name="const", bufs=1) as cpool,
            tc.tile_pool(name="xg", bufs=3) as xgp,
            tc.tile_pool(name="xt", bufs=2) as xtp,
            tc.tile_pool(name="cs", bufs=3) as csp,
            tc.tile_pool(name="as_", bufs=10) as asp,
            tc.tile_pool(name="qp", bufs=2) as qpp,
            tc.tile_pool(name="ag", bufs=8) as agp,
            tc.tile_pool(name="hg", bufs=2) as hgp,
            tc.tile_pool(name="sm", bufs=3) as smp,
            tc.tile_pool(name="pconv", bufs=2, space="PSUM") as pcv,
            tc.tile_pool(name="ptr", bufs=2, space="PSUM") as ptr,
            tc.tile_pool(name="pg", bufs=2, space="PSUM") as pgp,
            tc.tile_pool(name="pr", bufs=2, space="PSUM") as prp,
        ):
            idn = cpool.tile([128, 128], BF16)
            make_identity(nc, idn[:])
            qi = cpool.tile([128, BL], I32)
            nc.sync.dma_start(qi[:], qidx)
            ai = cpool.tile([128, 4 * BL], I32)
            nc.sync.dma_start(ai[:], aidx)
            w0 = cpool.tile([128, 1200], BF16)
            nc.sync.dma_start(w0[:], wc0)
            w1 = cpool.tile([128, 1200], BF16)
            nc.sync.dma_start(w1[:], wc1)
            w2 = cpool.tile([65, 1200], BF16)
            nc.sync.dma_start(w2[:], wc2)
            uu = cpool.tile([128, 2048], BF16)
            nc.sync.dma_start(uu[:], u_s)
            dot_acc = cpool.tile([1, BL], F32)
            q2_acc = cpool.tile([1, BL], F32)
            a2_acc = cpool.tile([1, BL], F32)

            for b in range(BL):
                # gather: block 0 = question, blocks 1..4 = answer chunks
                xg = xgp.tile([128, 5 * EP], BF16, tag="xg")
                nc.gpsimd.indirect_dma_start(
                    out=xg[:, 0:EP], out_offset=None, in_=emb,
                    in_offset=bass.IndirectOffsetOnAxis(ap=qi[:, b:b + 1], axis=0))
                for m in range(4):
                    nc.gpsimd.indirect_dma_start(
                        out=xg[:, (m + 1) * EP:(m + 2) * EP], out_offset=None,
                        in_=emb,
                        in_offset=bass.IndirectOffsetOnAxis(
                            ap=ai[:, 4 * b + m:4 * b + m + 1], axis=0))

                # transpose gathered [tokens, E] -> xT chunks [e, 643]:
                # col 0 = pad, 1:129 = question, 129 = pad, 130:642 = answer,
                # 642 = pad. Pads make every shifted conv window a full 128
                # cols so matmul outputs always span partitions 0:128.
                xt1 = xtp.tile([128, 643], BF16, tag="xt1")
                xt2 = xtp.tile([128, 643], BF16, tag="xt2")
                xt3 = xtp.tile([65, 643], BF16, tag="xt3")
                for xt in (xt1, xt2, xt3):
                    nc.any.memset(xt[:, 0:1], 0.0)
                    nc.any.memset(xt[:, 129:130], 0.0)
                    nc.any.memset(xt[:, 642:643], 0.0)
                for tb in range(5):
                    px = ptr.tile([128, 384], BF16, tag="ptr")
                    src = xg[:, tb * EP:(tb + 1) * EP]
                    nc.tensor.transpose(out=px[0:128, 0:128], in_=src[:, 0:128],
                                        identity=idn[:])
                    nc.tensor.transpose(out=px[0:128, 128:256], in_=src[:, 128:256],
                                        identity=idn[:])
                    nc.tensor.transpose(out=px[0:64, 256:384], in_=src[:, 256:320],
                                        identity=idn[:])
                    c0 = 1 + 128 * tb if tb == 0 else 130 + 128 * (tb - 1)
                    nc.any.tensor_copy(out=xt1[:, c0:c0 + 128], in_=px[0:128, 0:128])
                    nc.any.tensor_copy(out=xt2[:, c0:c0 + 128], in_=px[0:128, 128:256])
                    nc.any.tensor_copy(out=xt3[0:64, c0:c0 + 128], in_=px[0:64, 256:384])
                nc.any.memset(xt3[64:65, :], 1.0)

                # conv1d as shifted matmuls: out[l, f] = sum_{e,k} x[l+k-1, e] w_k[e, f]
                def conv_block(dst_ps, seg0):
                    first = True
                    for ec, xt, csz in ((0, xt1, 128), (1, xt2, 128), (2, xt3, 65)):
                        w = (w0, w1, w2)[ec]
                        for k in (0, 1, 2):
                            c = seg0 + k - 1
                            nc.tensor.matmul(
                                out=dst_ps[0:128, 0:400],
                                lhsT=xt[0:csz, c:c + 128],
                                rhs=w[0:csz, 400 * k:400 * k + 400],
                                start=first, stop=(ec == 2 and k == 2))
                            first = False

                qt_s = csp.tile([128, FP], BF16, tag="qt")
                pq = pcv.tile([128, 400], F32, tag="pconv")
                conv_block(pq, 1)
                nc.any.tensor_copy(out=qt_s[:, 0:400], in_=pq[:])
                nc.any.memset(qt_s[:, 400:512], 0.0)
                at_s = []
                for m in range(4):
                    pa = pcv.tile([128, 400], F32, tag="pconv")
                    conv_block(pa, 130 + 128 * m)
                    t = asp.tile([128, FP], BF16, tag="at")
                    nc.any.tensor_copy(out=t[:, 0:400], in_=pa[:])
                    nc.any.memset(t[:, 400:512], 0.0)
                    at_s.append(t)

                # transposes: QT -> Q_pack [f-chunk, q], AT -> A_g[j] [g-chunk, a]
                q_pack = qpp.tile([128, FP], BF16, tag="qpack")
                pt = ptr.tile([128, 512], BF16, tag="ptr")
                for j in range(4):
                    nc.tensor.transpose(out=pt[:, 128 * j:128 * j + 128],
                                        in_=qt_s[:, 128 * j:128 * j + 128],
                                        identity=idn[:])
                nc.any.tensor_copy(out=q_pack[:], in_=pt[:])
                a_g = []
                for _j in range(4):
                    agt = agp.tile([128, FP], BF16, tag="ag")
                    a_g.append(agt)
                for m in range(4):
                    pt2 = ptr.tile([128, 512], BF16, tag="ptr")
                    for j in range(4):
                        nc.tensor.transpose(out=pt2[:, 128 * j:128 * j + 128],
                                            in_=at_s[m][:, 128 * j:128 * j + 128],
                                            identity=idn[:])
                    for j in range(4):
                        nc.any.tensor_copy(out=a_g[j][:, 128 * m:128 * m + 128],
                                           in_=pt2[:, 128 * j:128 * j + 128])

                # H[g, q] = sum_f U[f, g] Q[f, q]
                ph = pgp.tile([128, 512], F32, tag="pg")
                for i in range(4):
                    for j in range(4):
                        nc.tensor.matmul(
                            out=ph[:, 128 * j:128 * j + 128],
                            lhsT=uu[:, 512 * i + 128 * j:512 * i + 128 * j + 128],
                            rhs=q_pack[:, 128 * i:128 * i + 128],
                            start=(i == 0), stop=(i == 3))
                h_s = hgp.tile([128, 512], BF16, tag="hs")
                nc.any.tensor_copy(out=h_s[:], in_=ph[:])

                # G[q, a] = sum_g H[g, q] A[g, a]
                pgt = pgp.tile([128, 512], F32, tag="pg")
                for j in range(4):
                    nc.tensor.matmul(out=pgt[:], lhsT=h_s[:, 128 * j:128 * j + 128],
                                     rhs=a_g[j][:], start=(j == 0), stop=(j == 3))
                mq = smp.tile([128, 1], F32, tag="mq")
                nc.vector.reduce_max(out=mq[:], in_=pgt[:], axis=AX)
                g_s = hgp.tile([128, 512], BF16, tag="gs")
                nc.any.tensor_copy(out=g_s[:], in_=pgt[:])

                # G^T blocks -> per-answer max
                pgt2 = ptr.tile([128, 512], BF16, tag="ptr")
                ma = smp.tile([128, 4], F32, tag="ma")
                for m in range(4):
                    nc.tensor.transpose(out=pgt2[:, 128 * m:128 * m + 128],
                                        in_=g_s[:, 128 * m:128 * m + 128],
                                        identity=idn[:])
                for m in range(4):
                    nc.vector.reduce_max(out=ma[:, m:m + 1],
                                         in_=pgt2[:, 128 * m:128 * m + 128], axis=AX)

                # attention weights: exp(tanh(max)) (softmax denom cancels in cosine)
                tq = smp.tile([128, 1], F32, tag="tq")
                eq = smp.tile([128, 1], BF16, tag="eq")
                nc.scalar.activation(out=tq[:], in_=mq[:], func=AF.Tanh)
                nc.scalar.activation(out=eq[:], in_=tq[:], func=AF.Exp)
                ta = smp.tile([128, 4], F32, tag="ta")
                ea = smp.tile([128, 4], BF16, tag="ea")
                nc.scalar.activation(out=ta[:], in_=ma[:], func=AF.Tanh)
                nc.scalar.activation(out=ea[:], in_=ta[:], func=AF.Exp)

                # pooled representations
                prq = prp.tile([1, 400], F32, tag="pr")
                nc.tensor.matmul(out=prq[:], lhsT=eq[:], rhs=qt_s[:, 0:400],
                                 start=True, stop=True)
                pra = prp.tile([1, 400], F32, tag="pr")
                for m in range(4):
                    nc.tensor.matmul(out=pra[:], lhsT=ea[:, m:m + 1],
                                     rhs=at_s[m][:, 0:400],
                                     start=(m == 0), stop=(m == 3))
                rq_s = smp.tile([1, 400], F32, tag="rqs")
                nc.any.tensor_copy(out=rq_s[:], in_=prq[:])
                prod = smp.tile([1, 400], F32, tag="prod")
                nc.vector.tensor_mul(out=prod[:], in0=rq_s[:], in1=pra[:])
                nc.vector.reduce_sum(out=dot_acc[0:1, b:b + 1], in_=prod[:], axis=AX)
                scr1 = smp.tile([1, 400], F32, tag="scr1")
                nc.scalar.activation(out=scr1[:], in_=prq[:], func=AF.Square,
                                     accum_out=q2_acc[0:1, b:b + 1])
                scr2 = smp.tile([1, 400], F32, tag="scr2")
                nc.scalar.activation(out=scr2[:], in_=pra[:], func=AF.Square,
                                     accum_out=a2_acc[0:1, b:b + 1])

            # cosine similarity finalize on [1, BL] vectors
            den = cpool.tile([1, BL], F32)
            nc.vector.tensor_mul(out=den[:], in0=q2_acc[:], in1=a2_acc[:])
            sq = cpool.tile([1, BL], F32)
            nc.scalar.activation(out=sq[:], in_=den[:], func=AF.Sqrt)
            inv = cpool.tile([1, BL], F32)
            nc.vector.reciprocal(out=inv[:], in_=sq[:])
            res = cpool.tile([1, BL], F32)
            nc.vector.tensor_mul(out=res[:], in0=dot_acc[:], in1=inv[:])
            nc.sync.dma_start(out_d.rearrange("(a b) -> a b", a=1), res[:])
    return nc


_BUILT = {}


def get_built():
    if "nc" not in _BUILT:
        nc = bacc.Bacc("TRN2", target_bir_lowering=False, debug=False,
                       num_devices=NCORES)
        build_kernel(nc)
        nc.compile()
        _BUILT["nc"] = nc
    return _BUILT["nc"]


def prep_inputs(question, answer, emb_table, conv_w, conv_b, U):
    bf = ml_dtypes.bfloat16
    emb_pad = np.zeros((V1, EP), dtype=bf)
    emb_pad[:, :E] = emb_table.astype(bf)

    wt = np.ascontiguousarray(conv_w.astype(np.float32).transpose(1, 0, 2))  # [E, F, K]
    wc0 = np.zeros((128, 1200), dtype=bf)
    wc1 = np.zeros((128, 1200), dtype=bf)
    wc2 = np.zeros((65, 1200), dtype=bf)
    for k in range(3):
        wc0[:, 400 * k:400 * k + 400] = wt[0:128, :, k].astype(bf)
        wc1[:, 400 * k:400 * k + 400] = wt[128:256, :, k].astype(bf)
        wc2[0:44, 400 * k:400 * k + 400] = wt[256:300, :, k].astype(bf)
    wc2[64, 400:800] = conv_b.astype(bf)  # bias row, k=1 block only

    u_pad = np.zeros((512, 512), dtype=np.float32)
    u_pad[:400, :400] = U.astype(np.float32)
    u_sh = np.zeros((128, 2048), dtype=bf)
    for i in range(4):
        u_sh[:, 512 * i:512 * i + 512] = u_pad[128 * i:128 * i + 128, :].astype(bf)

    qi = question.astype(np.int32)  # [B, 128]
    ai = answer.astype(np.int32)    # [B, 512]
    in_maps = []
    for c in range(NCORES):
        qs = qi[c * (B // NCORES):(c + 1) * (B // NCORES)][:BL]     # [BL, 128]
        as_ = ai[c * (B // NCORES):(c + 1) * (B // NCORES)][:BL]    # [BL, 512]
        qidx = np.ascontiguousarray(qs.T)                           # [128, BL]
        aidx = np.ascontiguousarray(
            as_.reshape(BL, 4, 128).transpose(2, 0, 1).reshape(128, 4 * BL))
        in_maps.append({
            "emb": emb_pad, "qidx": qidx, "aidx": aidx,
            "wc0": wc0, "wc1": wc1, "wc2": wc2, "u_s": u_sh,
        })
    return in_maps


def kernel(question, answer, emb_table, conv_w, conv_b, U):
    question = np.asarray(question)
    answer = np.asarray(answer)
    emb_table = np.asarray(emb_table, dtype=np.float32)
    conv_w = np.asarray(conv_w, dtype=np.float32)
    conv_b = np.asarray(conv_b, dtype=np.float32)
    U = np.asarray(U, dtype=np.float32)

    nc = get_built()
    in_maps = prep_inputs(question, answer, emb_table, conv_w, conv_b, U)
    res = bass_utils.run_bass_kernel_spmd(nc, in_maps, core_ids=list(range(NCORES)))
    out = np.concatenate([np.asarray(res.results[c]["out"]).reshape(-1)
                          for c in range(NCORES)])
    return out.astype(np.float32)


# revision 23
# speedup vs baseline: 95.9954x; 95.9954x over previous
"""AttentivePoolingNetwork Trainium2 kernel.

Data-parallel over batch across 8 NeuronCores (64 batch elements each).
Per batch element, fully fused on-chip:
  gather bf16 emb rows -> PE-transpose to [E, L] -> conv1d(k=3) as shifted
  matmuls (bias folded in via ones-row) -> QT/AT [tokens, F] -> transposes
  -> H = U^T Q -> G = H^T A -> row/col maxes -> exp(tanh(max)) weights
  (softmax denominators cancel in the final cosine similarity) -> pooled
  rQ/rA via tiny matmuls -> cosine similarity per element.
"""

import os
import numpy as np
import ml_dtypes

import concourse.bacc as bacc
import concourse.bass as bass
import concourse.tile as tile
import concourse.mybir as mybir
from concourse import bass_utils
from concourse.masks import make_identity

BF16 = mybir.dt.bfloat16
F32 = mybir.dt.float32
I32 = mybir.dt.int32
AX = mybir.AxisListType.X
AF = mybir.ActivationFunctionType

B, QL, AL = 512, 128, 512
V1, E, F = 50001, 300, 400
NCORES = 8
BL = int(os.environ.get("KBL", B // NCORES))  # batch elems per core
ABL = set(os.environ.get("ABL", "").split(","))  # ablation flags (timing expts)
EP = 320   # emb width padded (300 -> 320), bf16 rows = 640B
FP = 512   # feature width padded (400 -> 512)


def build_kernel(nc):
    emb = nc.dram_tensor("emb", [V1, EP], BF16, kind="ExternalInput").ap()
    qidx = nc.dram_tensor("qidx", [128, BL], I32, kind="ExternalInput").ap()
    aidx = nc.dram_tensor("aidx", [128, 4 * BL], I32, kind="ExternalInput").ap()
    wc0 = nc.dram_tensor("wc0", [128, 1200], BF16, kind="ExternalInput").ap()
    wc1 = nc.dram_tensor("wc1", [128, 1200], BF16, kind="ExternalInput").ap()
    wc2 = nc.dram_tensor("wc2", [65, 1200], BF16, kind="ExternalInput").ap()
    u_s = nc.dram_tensor("u_s", [128, 2048], BF16, kind="ExternalInput").ap()
    out_d = nc.dram_tensor("out", [BL], F32, kind="ExternalOutput").ap()

    with tile.TileContext(nc) as tc:
        with (
            tc.tile_pool(name="const", bufs=1) as cpool,
            tc.tile_pool(name="xg", bufs=3) as xgp,
            tc.tile_pool(name="xt", bufs=2) as xtp,
            tc.tile_pool(name="cs", bufs=3) as csp,
            tc.tile_pool(name="as_", bufs=10) as asp,
            tc.tile_pool(name="qp", bufs=2) as qpp,
            tc.tile_pool(name="ag", bufs=8) as agp,
            tc.tile_pool(name="hg", bufs=2) as hgp,
            tc.tile_pool(name="sm", bufs=3) as smp,
            tc.tile_pool(name="pconv", bufs=2, space="PSUM") as pcv,
            tc.tile_pool(name="ptr", bufs=2, space="PSUM") as ptr,
            tc.tile_pool(name="pg", bufs=2, space="PSUM") as pgp,
            tc.tile_pool(name="pr", bufs=2, space="PSUM") as prp,
        ):
            idn = cpool.tile([128, 128], BF16)
            make_identity(nc, idn[:])
            qi = cpool.tile([128, BL], I32)
            nc.sync.dma_start(qi[:], qidx)
            ai = cpool.tile([128, 4 * BL], I32)
            nc.sync.dma_start(ai[:], aidx)
            w0 = cpool.tile([128, 1200], BF16)
            nc.sync.dma_start(w0[:], wc0)
            w1 = cpool.tile([128, 1200], BF16)
            nc.sync.dma_start(w1[:], wc1)
            w2 = cpool.tile([65, 1200], BF16)
            nc.sync.dma_start(w2[:], wc2)
            uu = cpool.tile([128, 2048], BF16)
            nc.sync.dma_start(uu[:], u_s)
            dot_acc = cpool.tile([1, BL], F32)
            q2_acc = cpool.tile([1, BL], F32)
            a2_acc = cpool.tile([1, BL], F32)

            def emit_tail(b, g_s, qt_s, at_s, eq):
                pgt2 = ptr.tile([128, 512], BF16, tag="ptr")
                ma = smp.tile([128, 4], F32, tag="ma")
                for m in range(4):
                    nc.tensor.transpose(out=pgt2[:, 128 * m:128 * m + 128],
                                        in_=g_s[:, 128 * m:128 * m + 128],
                                        identity=idn[:])
                for m in range(4):
                    nc.vector.reduce_max(out=ma[:, m:m + 1],
                                         in_=pgt2[:, 128 * m:128 * m + 128], axis=AX)
                ta = smp.tile([128, 4], F32, tag="ta")
                ea = smp.tile([128, 4], BF16, tag="ea")
                nc.scalar.activation(out=ta[:], in_=ma[:], func=AF.Tanh)
                nc.scalar.activation(out=ea[:], in_=ta[:], func=AF.Exp)
                prq = prp.tile([1, 400], F32, tag="pr")
                nc.tensor.matmul(out=prq[:], lhsT=eq[:], rhs=qt_s[:, 0:400],
                                 start=True, stop=True)
                pra = prp.tile([1, 400], F32, tag="pr")
                for m in range(4):
                    nc.tensor.matmul(out=pra[:], lhsT=ea[:, m:m + 1],
                                     rhs=at_s[m][:, 0:400],
                                     start=(m == 0), stop=(m == 3))
                rq_s = smp.tile([1, 400], F32, tag="rqs")
                nc.any.tensor_copy(out=rq_s[:], in_=prq[:])
                prod = smp.tile([1, 400], F32, tag="prod")
                nc.vector.tensor_mul(out=prod[:], in0=rq_s[:], in1=pra[:])
                nc.vector.reduce_sum(out=dot_acc[0:1, b:b + 1], in_=prod[:], axis=AX)
                scr1 = smp.tile([1, 400], F32, tag="scr1")
                nc.scalar.activation(out=scr1[:], in_=prq[:], func=AF.Square,
                                     accum_out=q2_acc[0:1, b:b + 1])
                scr2 = smp.tile([1, 400], F32, tag="scr2")
                nc.scalar.activation(out=scr2[:], in_=pra[:], func=AF.Square,
                                     accum_out=a2_acc[0:1, b:b + 1])

            pending = []
            PIPE = int(os.environ.get("PIPE", 1))
            for b in range(BL):
                # gather: block 0 = question, blocks 1..4 = answer chunks
                xg = xgp.tile([128, 5 * EP], BF16, tag="xg")
                if "gather" not in ABL:
                  nc.gpsimd.indirect_dma_start(
                    out=xg[:, 0:EP], out_offset=None, in_=emb,
                    in_offset=bass.IndirectOffsetOnAxis(ap=qi[:, b:b + 1], axis=0))
                  for m in range(4):
                    nc.gpsimd.indirect_dma_start(
                        out=xg[:, (m + 1) * EP:(m + 2) * EP], out_offset=None,
                        in_=emb,
                        in_offset=bass.IndirectOffsetOnAxis(
                            ap=ai[:, 4 * b + m:4 * b + m + 1], axis=0))

                # transpose gathered [tokens, E] -> xT chunks [e, 643]:
                # col 0 = pad, 1:129 = question, 129 = pad, 130:642 = answer,
                # 642 = pad. Pads make every shifted conv window a full 128
                # cols so matmul outputs always span partitions 0:128.
                xt1 = xtp.tile([128, 643], BF16, tag="xt1")
                xt2 = xtp.tile([128, 643], BF16, tag="xt2")
                xt3 = xtp.tile([65, 643], BF16, tag="xt3")
                for xt in (xt1, xt2, xt3):
                    nc.any.memset(xt[:, 0:1], 0.0)
                    nc.any.memset(xt[:, 129:130], 0.0)
                    nc.any.memset(xt[:, 642:643], 0.0)
                nc.any.memset(xt3[64:65, :], 1.0)

                def emit_xt(tb):
                    px = ptr.tile([128, 384], BF16, tag="ptr")
                    src = xg[:, tb * EP:(tb + 1) * EP]
                    nc.tensor.transpose(out=px[0:128, 0:128], in_=src[:, 0:128],
                                        identity=idn[:])
                    nc.tensor.transpose(out=px[0:128, 128:256], in_=src[:, 128:256],
                                        identity=idn[:])
                    nc.tensor.transpose(out=px[0:64, 256:384], in_=src[:, 256:320],
                                        identity=idn[:])
                    c0 = 1 + 128 * tb if tb == 0 else 130 + 128 * (tb - 1)
                    nc.any.tensor_copy(out=xt1[:, c0:c0 + 128], in_=px[0:128, 0:128])
                    nc.any.tensor_copy(out=xt2[:, c0:c0 + 128], in_=px[0:128, 128:256])
                    nc.any.tensor_copy(out=xt3[0:64, c0:c0 + 128], in_=px[0:64, 256:384])

                # conv1d as shifted matmuls: out[l, f] = sum_{e,k} x[l+k-1, e] w_k[e, f]
                def conv_block(dst_ps, seg0):
                    first = True
                    for ec, xt, csz in ((0, xt1, 128), (1, xt2, 128), (2, xt3, 65)):
                        w = (w0, w1, w2)[ec]
                        for k in (0, 1, 2):
                            c = seg0 + k - 1
                            nc.tensor.matmul(
                                out=dst_ps[0:128, 0:400],
                                lhsT=xt[0:csz, c:c + 128],
                                rhs=w[0:csz, 400 * k:400 * k + 400],
                                start=first, stop=(ec == 2 and k == 2))
                            first = False

                for tb in range(5) if "xt" not in ABL else []:
                    emit_xt(tb)
                qt_s = csp.tile([128, FP], BF16, tag="qt")
                pq = pcv.tile([128, 400], F32, tag="pconv")
                if "conv" not in ABL:
                    conv_block(pq, 1)
                nc.any.tensor_copy(out=qt_s[:, 0:400], in_=pq[:])
                nc.any.memset(qt_s[:, 400:512], 0.0)
                at_s = []
                for m in range(4):
                    pa = pcv.tile([128, 400], F32, tag="pconv")
                    if "conv" not in ABL:
                        conv_block(pa, 130 + 128 * m)
                    t = asp.tile([128, FP], BF16, tag="at")
                    nc.any.tensor_copy(out=t[:, 0:400], in_=pa[:])
                    nc.any.memset(t[:, 400:512], 0.0)
                    at_s.append(t)

                # transposes: QT -> Q_pack [f-chunk, q], AT -> A_g[j] [g-chunk, a]
                q_pack = qpp.tile([128, FP], BF16, tag="qpack")
                pt = ptr.tile([128, 512], BF16, tag="ptr")
                for j in range(4) if "qat" not in ABL else []:
                    nc.tensor.transpose(out=pt[:, 128 * j:128 * j + 128],
                                        in_=qt_s[:, 128 * j:128 * j + 128],
                                        identity=idn[:])
                nc.any.tensor_copy(out=q_pack[:], in_=pt[:])
                def emit_at2(m):
                    pt2 = ptr.tile([128, 512], BF16, tag="ptr")
                    for j in range(4):
                        nc.tensor.transpose(out=pt2[:, 128 * j:128 * j + 128],
                                            in_=at_s[m][:, 128 * j:128 * j + 128],
                                            identity=idn[:])
                    agt = agp.tile([128, FP], BF16, tag="ag")
                    nc.any.tensor_copy(out=agt[:], in_=pt2[:])
                    a_t2.append(agt)

                a_t2 = []
                for m in range(3) if "qat" not in ABL else []:
                    emit_at2(m)

                # H[g, q] = sum_f U[f, g] Q[f, q]  (emitted before the last
                # A-transpose so its copy drains under H's matmuls)
                ph = pgp.tile([128, 512], F32, tag="pg")
                for i in range(4) if "ug" not in ABL else []:
                    for j in range(4):
                        nc.tensor.matmul(
                            out=ph[:, 128 * j:128 * j + 128],
                            lhsT=uu[:, 512 * i + 128 * j:512 * i + 128 * j + 128],
                            rhs=q_pack[:, 128 * i:128 * i + 128],
                            start=(i == 0), stop=(i == 3))
                if "qat" not in ABL:
                    emit_at2(3)
                h_s = hgp.tile([128, 512], BF16, tag="hs")
                nc.any.tensor_copy(out=h_s[:], in_=ph[:])

                # G[q, a] = sum_g H[g, q] A[g, a]
                pgt = pgp.tile([128, 512], F32, tag="pg")
                for m in range(4) if "ug" not in ABL else []:
                    for j in range(4):
                        nc.tensor.matmul(
                            out=pgt[:, 128 * m:128 * m + 128],
                            lhsT=h_s[:, 128 * j:128 * j + 128],
                            rhs=a_t2[m][:, 128 * j:128 * j + 128],
                            start=(j == 0), stop=(j == 3))
                mq = smp.tile([128, 1], F32, tag="mq")
                nc.vector.reduce_max(out=mq[:], in_=pgt[:], axis=AX)
                g_s = hgp.tile([128, 512], BF16, tag="gs")
                nc.any.tensor_copy(out=g_s[:], in_=pgt[:])
                tq = smp.tile([128, 1], F32, tag="tq")
                eq = smp.tile([128, 1], BF16, tag="eq")
                nc.scalar.activation(out=tq[:], in_=mq[:], func=AF.Tanh)
                nc.scalar.activation(out=eq[:], in_=tq[:], func=AF.Exp)

                # defer the tail (G^T maxes + pooling) one iteration so its
                # cross-engine operands are ready when the in-order PE queue
                # reaches it (software pipelining of the PE stall).
                pending.append((b, g_s, qt_s, at_s, eq))
                if len(pending) > PIPE:
                    emit_tail(*pending.pop(0))

            for p in pending:
                emit_tail(*p)

            # cosine similarity finalize on [1, BL] vectors
            den = cpool.tile([1, BL], F32)
            nc.vector.tensor_mul(out=den[:], in0=q2_acc[:], in1=a2_acc[:])
            sq = cpool.tile([1, BL], F32)
            nc.scalar.activation(out=sq[:], in_=den[:], func=AF.Sqrt)
            inv = cpool.tile([1, BL], F32)
            nc.vector.reciprocal(out=inv[:], in_=sq[:])
            res = cpool.tile([1, BL], F32)
            nc.vector.tensor_mul(out=res[:], in0=dot_acc[:], in1=inv[:])
            nc.sync.dma_start(out_d.rearrange("(a b) -> a b", a=1), res[:])
    return nc


_BUILT = {}


def get_built():
    if "nc" not in _BUILT:
        nc = bacc.Bacc("TRN2", target_bir_lowering=False, debug=False,
                       num_devices=NCORES)
        build_kernel(nc)
        nc.compile()
        _BUILT["nc"] = nc
    return _BUILT["nc"]


def prep_inputs(question, answer, emb_table, conv_w, conv_b, U):
    bf = ml_dtypes.bfloat16
    emb_pad = np.zeros((V1, EP), dtype=bf)
    emb_pad[:, :E] = emb_table.astype(bf)

    wt = np.ascontiguousarray(conv_w.astype(np.float32).transpose(1, 0, 2))  # [E, F, K]
    wc0 = np.zeros((128, 1200), dtype=bf)
    wc1 = np.zeros((128, 1200), dtype=bf)
    wc2 = np.zeros((65, 1200), dtype=bf)
    for k in range(3):
        wc0[:, 400 * k:400 * k + 400] = wt[0:128, :, k].astype(bf)
        wc1[:, 400 * k:400 * k + 400] = wt[128:256, :, k].astype(bf)
        wc2[0:44, 400 * k:400 * k + 400] = wt[256:300, :, k].astype(bf)
    wc2[64, 400:800] = conv_b.astype(bf)  # bias row, k=1 block only

    u_pad = np.zeros((512, 512), dtype=np.float32)
    u_pad[:400, :400] = U.astype(np.float32)
    u_sh = np.zeros((128, 2048), dtype=bf)
    for i in range(4):
        u_sh[:, 512 * i:512 * i + 512] = u_pad[128 * i:128 * i + 128, :].astype(bf)

    qi = question.astype(np.int32)  # [B, 128]
    ai = answer.astype(np.int32)    # [B, 512]
    in_maps = []
    for c in range(NCORES):
        qs = qi[c * (B // NCORES):(c + 1) * (B // NCORES)][:BL]     # [BL, 128]
        as_ = ai[c * (B // NCORES):(c + 1) * (B // NCORES)][:BL]    # [BL, 512]
        qidx = np.ascontiguousarray(qs.T)                           # [128, BL]
        aidx = np.ascontiguousarray(
            as_.reshape(BL, 4, 128).transpose(2, 0, 1).reshape(128, 4 * BL))
        in_maps.append({
            "emb": emb_pad, "qidx": qidx, "aidx": aidx,
            "wc0": wc0, "wc1": wc1, "wc2": wc2, "u_s": u_sh,
        })
    return in_maps


def kernel(question, answer, emb_table, conv_w, conv_b, U):
    question = np.asarray(question)
    answer = np.asarray(answer)
    emb_table = np.asarray(emb_table, dtype=np.float32)
    conv_w = np.asarray(conv_w, dtype=np.float32)
    conv_b = np.asarray(conv_b, dtype=np.float32)
    U = np.asarray(U, dtype=np.float32)

    nc = get_built()
    in_maps = prep_inputs(question, answer, emb_table, conv_w, conv_b, U)
    res = bass_utils.run_bass_kernel_spmd(nc, in_maps, core_ids=list(range(NCORES)))
    out = np.concatenate([np.asarray(res.results[c]["out"]).reshape(-1)
                          for c in range(NCORES)])
    return out.astype(np.float32)
